# revision 1
# baseline (speedup 1.0000x reference)
"""Direct-Form-II biquad (order-2 IIR) over [B=64, T=262144, 1] on 8 trn2 cores.

Algorithm
---------
The recurrence
    y[t] = b0 x[t] + b1 x[t-1] + b2 x[t-2] - a1 y[t-1] - a2 y[t-2]
is a linear time-invariant filter whose impulse response g decays
geometrically (|poles| < 1 for the sampled coefficients), so to fp32
precision the IIR equals a short FIR: y = conv(x, g[:K]).

On device the FIR is computed with the tensor engine in overlap-save form.
Per sequence, x is laid out in SBUF as [128 partitions, 2048] with partition
p holding x[p*2048 : (p+1)*2048] (contiguous DMA). Each 128x128 tile of that
layout holds 128 chunks (partitions = chunk index c = p*16 + f1, free =
within-chunk time j). Tiles are PE-transposed so j lands on partitions, then
one matmul per tile, with the transposed tile as the stationary operand and a
fused [A^T | B^T] Toeplitz coefficient block as the moving operand, produces
the within-chunk FIR term (A-half) and the spill-over into the next chunk
(B-half). ys[f1] = A(f1) + B(f1-1) is assembled during PSUM evacuation:
an A-copy (ACT/DVE) plus a read-modify-write B-add (DVE) — PSUM has a single
DVE read port, so the two PSUM halves are never read by one instruction.

Sharding: pure data parallelism, batch 64 -> 8 sequences per core.
"""

import os
from contextlib import ExitStack

import numpy as np

_B, _T = 64, 262144
_NCORES = 8
_S = _B // _NCORES          # sequences per core
_P = 128                    # partitions / chunk length
_F = _T // _P               # 2048 free columns per sequence
_NT = _F // _P              # 16 tiles per sequence

# 'fp32'  : exact fp32 matmuls (4 cycles/row on PE)
# 'f32r'  : rounded fp32 (12-bit mantissa) matmuls at full PE rate
_MODE = os.environ.get("BIQUAD_MODE", "bf16")

_runner_cache = {}


def _impulse_response(b0, b1, b2, a1, a2, n):
    """Float64 impulse response of the reference recurrence."""
    g = np.zeros(n, dtype=np.float64)
    v0 = 0.0
    v1 = 0.0
    for t in range(n):
        xt = 1.0 if t == 0 else 0.0
        out = xt * b0 + v0
        v0_new = xt * b1 + v1 - out * a1
        v1_new = xt * b2 - out * a2
        v0, v1 = v0_new, v1_new
        g[t] = out
    return g


def _coef_block(g, kb):
    """[128, 128 + kb] moving operand: columns = output offset i.

    A^T[j, i] = g[i - j]          (within-chunk taps, i in [0,128))
    B^T[j, i] = g[i + 128 - j]    (taps reaching one chunk back, i in [0,kb))
    """
    K = len(g)
    A = np.zeros((_P, _P), dtype=np.float64)
    Bm = np.zeros((_P, kb), dtype=np.float64)
    for j in range(_P):
        for i in range(_P):
            if 0 <= i - j < K:
                A[j, i] = g[i - j]
        for i in range(kb):
            k = i + _P - j
            if 0 <= k < K:
                Bm[j, i] = g[k]
    return np.concatenate([A, Bm], axis=1).astype(np.float32)


def _build_bf16(kb, repeat=1):
    """bf16 pipeline:

    - input arrives as bf16 via gpsimd cast-DMA (the SW DGE casts f32 ->
      bf16 in the DMA path at full ~400 GB/s fabric rate), all 8 loads
      issued up front into 8 resident tiles;
    - PE transposes each [128,128] tile (bf16, 1 cycle/row) into PSUM,
      ACT/DVE evacuate into the xt tile;
    - FIR = per tile an A-matmul (coef cols 0:128) plus a B-matmul of the
      PREVIOUS tile (coef cols 128:128+kb) accumulated into the same PSUM
      slot, so each y psum group is final: evacuation is a single
      contiguous copy and there is no separate b_add pass;
    - the loop is software-pipelined: transposes of sequence s run before
      the FIR of sequence s-1 in PE program order, keeping the PE busy
      (and its p-state ramped) while evacuations catch up.
    """
    from concourse import bacc, mybir, tile

    nc = bacc.Bacc("TRN2", target_bir_lowering=False, debug=False)
    f32 = mybir.dt.float32
    bf16 = mybir.dt.bfloat16

    NC = _P + kb
    x_d = nc.dram_tensor("x", [_S, _P, _F], f32, kind="ExternalInput")
    coef_d = nc.dram_tensor("coef", [_P, NC], f32, kind="ExternalInput")
    id_d = nc.dram_tensor("ident", [_P, _P], f32, kind="ExternalInput")
    y_d = nc.dram_tensor("y", [_S, _P, _F], f32, kind="ExternalOutput")

    XT_SLOTS = _NT + 1                # 16 transposed tiles + shifted m1 tile
    TRG = 8                           # transposes per psum tile (1 bank bf16)

    with tile.TileContext(nc) as tc, ExitStack() as ctx:
        cpool = ctx.enter_context(tc.tile_pool(name="consts", bufs=1))
        xbpool = ctx.enter_context(tc.tile_pool(name="xb", bufs=1))
        xtpool = ctx.enter_context(tc.tile_pool(name="xt", bufs=2))
        ypool = ctx.enter_context(tc.tile_pool(name="yout", bufs=3))
        ptp = ctx.enter_context(tc.tile_pool(name="pt", bufs=3, space="PSUM"))
        pyp = ctx.enter_context(tc.tile_pool(name="py", bufs=5, space="PSUM"))

        id_sb = cpool.tile([_P, _P], f32)
        nc.sync.dma_start(id_sb[:], id_d.ap())
        coef_sb = cpool.tile([_P, NC], f32)
        nc.sync.dma_start(coef_sb[:], coef_d.ap())
        coef_c = cpool.tile([_P, NC], bf16)
        nc.vector.tensor_copy(coef_c[:], coef_sb[:])
        id_c = cpool.tile([_P, _P], bf16)
        nc.vector.tensor_copy(id_c[:], id_sb[:])

        # sync is otherwise idle: give it every output DMA so the busy
        # ACT engine never pays the ~0.7us HWDGE issue cost
        out_eng = [nc.sync, nc.scalar] if os.environ.get("OUT_SPLIT") == "1" else [nc.sync, nc.sync]
        xt_eng = [nc.scalar.copy, nc.vector.tensor_copy]
        a_eng = [nc.scalar.copy, nc.vector.tensor_copy,
                 nc.scalar.copy, nc.vector.tensor_copy]

        from contextlib import nullcontext
        loop_ctx = tc.For_i(0, repeat, 1) if repeat > 1 else nullcontext()
        with loop_ctx:
            xbs = [xbpool.tile([_P, _F], bf16, name=f"xb{s}")
                   for s in range(_S)]
            for s in range(_S):
                nc.gpsimd.dma_start(xbs[s][:], x_d.ap()[s])

            def transpose_stage(s):
                xb = xbs[s]
                # natural order -> each psum group evacuates in ONE copy;
                # the m1 tile only feeds the NEXT pipeline stage's FIR, so
                # tile 15 no longer needs to come first
                perm = list(range(16))
                xt = xtpool.tile([_P, XT_SLOTS * _P], bf16, name="xt")
                for gidx in range(16 // TRG):
                    ptile = ptp.tile([_P, TRG * _P], bf16, name="pt")
                    grp = perm[TRG * gidx : TRG * gidx + TRG]
                    for q, f1 in enumerate(grp):
                        nc.tensor.transpose(
                            ptile[:, q * _P : (q + 1) * _P],
                            xb[:, f1 * _P : (f1 + 1) * _P],
                            id_c[:],
                        )
                    q0 = 0
                    while q0 < TRG:
                        q1 = q0 + 1
                        while q1 < TRG and grp[q1] == grp[q1 - 1] + 1:
                            q1 += 1
                        xt_eng[gidx](
                            xt[:, grp[q0] * _P : (grp[q0] + q1 - q0) * _P],
                            ptile[:, q0 * _P : q1 * _P],
                        )
                        q0 = q1
                # m1 boundary tile: m1[col p] = tile15[col p-1], col 0 = 0
                m1 = _NT * _P
                nc.vector.memset(xt[:, m1 : m1 + 1], 0.0)
                nc.gpsimd.tensor_copy(
                    xt[:, m1 + 1 : m1 + _P],
                    xt[:, 15 * _P : 16 * _P - 1],
                )
                return xt

            def fir_stage(s, xt):
                # 4 psum groups of 4 final y tiles each; per tile an
                # A-matmul then an accumulated B-matmul of the previous
                # tile (m1 slot for tile 0)
                ys = ypool.tile([_P, _F], f32, name="ys")
                for g in range(4):
                    pt_ = pyp.tile([_P, 4 * _P], f32, name="py")
                    for q in range(4):
                        t = 4 * g + q
                        prev = _NT if t == 0 else t - 1
                        nc.tensor.matmul(
                            pt_[:, q * _P : q * _P + _P],
                            xt[:, t * _P : (t + 1) * _P],
                            coef_c[:, 0:_P],
                            start=True,
                            stop=False,
                        )
                        nc.tensor.matmul(
                            pt_[:, q * _P : q * _P + kb],
                            xt[:, prev * _P : (prev + 1) * _P],
                            coef_c[:, _P : _P + kb],
                            start=False,
                            stop=True,
                        )
                    a_eng[g](ys[:, g * 512 : (g + 1) * 512], pt_[:])
                out_eng[s % 2].dma_start(y_d.ap()[s], ys[:])

            prev_xt = None
            for s in range(_S):
                xt = transpose_stage(s)
                if prev_xt is not None:
                    fir_stage(s - 1, prev_xt)
                prev_xt = xt
            fir_stage(_S - 1, prev_xt)

    nc.compile()
    return nc


def _build_program(mode, kb, repeat=1):
    if mode == "bf16":
        return _build_bf16(kb, repeat)

    from concourse import bacc, mybir, tile

    nc = bacc.Bacc("TRN2", target_bir_lowering=False, debug=False)
    f32 = mybir.dt.float32
    cdt = {
        "fp32": f32,
        "f32r": mybir.dt.float32r,
    }[mode]

    # f32r matmuls only run at full PE rate when the moving operand is
    # >= 256 columns, so the coef block is zero-padded to NC=256 there;
    # b_add still only touches the true kb columns.
    kb_mm = 128 if mode == "f32r" else kb
    NC = _P + kb_mm                   # moving operand width
    SLOT = 256 if NC <= 256 else 512  # psum slot stride (bank-crossing safe)
    x_d = nc.dram_tensor("x", [_S, _P, _F], f32, kind="ExternalInput")
    coef_d = nc.dram_tensor("coef", [_P, NC], f32, kind="ExternalInput")
    id_d = nc.dram_tensor("ident", [_P, _P], f32, kind="ExternalInput")
    y_d = nc.dram_tensor("y", [_S, _P, _F], f32, kind="ExternalOutput")

    XT_SLOTS = _NT + 1                # 16 transposed tiles + shifted m1 tile

    with tile.TileContext(nc) as tc, ExitStack() as ctx:
        cpool = ctx.enter_context(tc.tile_pool(name="consts", bufs=1))
        xpool = ctx.enter_context(tc.tile_pool(name="xin", bufs=2))
        xtpool = ctx.enter_context(tc.tile_pool(name="xt", bufs=2))
        ypool = ctx.enter_context(tc.tile_pool(name="yout", bufs=2))
        ptp = ctx.enter_context(tc.tile_pool(name="pt", bufs=2, space="PSUM"))
        pyp = ctx.enter_context(tc.tile_pool(name="py", bufs=3, space="PSUM"))
        if mode == "bf16":
            # bf16 input tiles are 0.5 MiB; 8 distinct tiles (bufs=1 each)
            # buffer the whole input so the cast-DMA stream runs
            # back-to-back at full fabric rate instead of being gated by
            # compute retiring buffers.
            xbpool = ctx.enter_context(tc.tile_pool(name="xb", bufs=1))

        id_sb = cpool.tile([_P, _P], f32)
        nc.sync.dma_start(id_sb[:], id_d.ap())
        coef_sb = cpool.tile([_P, NC], f32)
        nc.sync.dma_start(coef_sb[:], coef_d.ap())
        if mode != "fp32":
            coef_c = cpool.tile([_P, NC], cdt)
            nc.vector.tensor_copy(coef_c[:], coef_sb[:])
        else:
            coef_c = coef_sb
        if mode == "bf16":
            id_c = cpool.tile([_P, _P], cdt)
            nc.vector.tensor_copy(id_c[:], id_sb[:])
        else:
            id_c = id_sb
        if mode == "f32r":
            # f32r memset fails the ISA check on every engine; keep an f32
            # zero column and round it through an ACT copy where needed.
            zcol = cpool.tile([_P, 1], f32)
            nc.vector.memset(zcol[:], 0.0)

        # DMA queue assignment: only sync (HW DGE), scalar (HW DGE) and
        # gpsimd (SW DGE) can issue DMAs.  A single queue saturates the
        # ~420 GB/s fabric when its transfers are back-to-back, so the
        # split below is about overlap, not per-queue bandwidth.  In bf16
        # mode the input ride gpsimd's SW DGE, which is the only queue
        # that can cast f32 -> bf16 in the DMA path (measured at the same
        # ~400 GB/s as a plain load - the conversion is free).
        in_eng = [nc.sync, nc.scalar]
        out_eng = [nc.sync, nc.scalar] if mode == "bf16" else [
            nc.gpsimd, nc.sync, nc.scalar]

        # engine for the xt evacuation copies, per transpose psum group
        if mode == "bf16":
            xt_eng = [nc.scalar.copy, nc.vector.tensor_copy]
        else:
            xt_eng = [nc.vector.tensor_copy, nc.scalar.copy,
                      nc.vector.tensor_copy, nc.scalar.copy]

        from contextlib import nullcontext
        loop_ctx = tc.For_i(0, repeat, 1) if repeat > 1 else nullcontext()
        with loop_ctx:
          if mode == "bf16":
              # issue all 8 cast-DMA input loads up front: gpsimd's
              # instruction stream has no compute before them, so the
              # input streams back-to-back at full fabric rate
              xbs = [xbpool.tile([_P, _F], cdt, name=f"xb{s}")
                     for s in range(_S)]
              for s in range(_S):
                  nc.gpsimd.dma_start(xbs[s][:], x_d.ap()[s])
          for s in range(_S):
            # --- load x[s] as [128, 2048], partition p = x[p*2048 + f] ---
            # one 1 MiB dma_start (max-bandwidth shape: 128 partitions, >=1MiB)
            if mode == "bf16":
                tr_src, tr_id = xbs[s], id_c[:]
            else:
                xs = xpool.tile([_P, _F], f32)
                in_eng[s % len(in_eng)].dma_start(xs[:], x_d.ap()[s])
                tr_src, tr_id = xs, id_sb[:]

            # --- PE transposes + engine evacuation ---
            # Tile 15 is transposed first so the m1 boundary tile (and its
            # matmul) can run early, keeping all PSUM pairs short-lived.
            perm = [15] + list(range(15))
            xt = xtpool.tile([_P, XT_SLOTS * _P], cdt)
            tdt = cdt if mode == "bf16" else f32
            # bf16 psum tiles are 2 bytes/col: 8 transposes fit one bank,
            # halving the number of (latency-dominated) evacuation copies
            TRG = 8 if mode == "bf16" else 4
            for gidx in range(16 // TRG):
                ptile = ptp.tile([_P, TRG * _P], tdt)
                grp = perm[TRG * gidx : TRG * gidx + TRG]
                for q, f1 in enumerate(grp):
                    nc.tensor.transpose(
                        ptile[:, q * _P : (q + 1) * _P],
                        tr_src[:, f1 * _P : (f1 + 1) * _P],
                        tr_id,
                    )
                # copy contiguous runs of the permuted group into xt
                q0 = 0
                while q0 < TRG:
                    q1 = q0 + 1
                    while q1 < TRG and grp[q1] == grp[q1 - 1] + 1:
                        q1 += 1
                    xt_eng[gidx](
                        xt[:, grp[q0] * _P : (grp[q0] + q1 - q0) * _P],
                        ptile[:, q0 * _P : q1 * _P],
                    )
                    q0 = q1

            # --- m1 boundary tile: m1[col p] = tile15[col p-1], col 0 = 0 ---
            m1 = _NT * _P
            if mode == "f32r":
                # f32r memset is rejected by the ISA checker; round an f32
                # zero through ACT instead, and shift-copy on DVE.
                nc.scalar.copy(xt[:, m1 : m1 + 1], zcol[:])
                nc.vector.tensor_copy(
                    xt[:, m1 + 1 : m1 + _P],
                    xt[:, 15 * _P : 16 * _P - 1],
                )
            elif mode == "bf16":
                # gpsimd is free after the hoisted input DMAs (SBUF-only
                # ops; gpsimd cannot touch PSUM)
                nc.vector.memset(xt[:, m1 : m1 + 1], 0.0)
                nc.gpsimd.tensor_copy(
                    xt[:, m1 + 1 : m1 + _P],
                    xt[:, 15 * _P : 16 * _P - 1],
                )
            else:
                nc.gpsimd.memset(xt[:, m1 : m1 + 1], 0.0)
                nc.gpsimd.tensor_copy(
                    xt[:, m1 + 1 : m1 + _P],
                    xt[:, 15 * _P : 16 * _P - 1],
                )

            # --- matmuls (fused [A|B] moving operand) + batched evacuation ---
            # Groups of 4 matmuls per PSUM tile:
            #   G0=[m1,t0,t1,t2] G1=[t3..6] G2=[t7..10] G3=[t11..14] G4=[t15]
            # ys[f1] = A-half(f1) + B-half(f1-1); m1's B-half feeds ys[0].
            ys = ypool.tile([_P, _F], f32)
            groups = [[_NT, 0, 1, 2], [3, 4, 5, 6], [7, 8, 9, 10],
                      [11, 12, 13, 14], [15]]
            ptiles = []

            def mm_group(gi):
                grp = groups[gi]
                pt_ = pyp.tile([_P, 4 * SLOT], f32, tag="py")
                for k, idx in enumerate(grp):
                    nc.tensor.matmul(
                        pt_[:, k * SLOT : k * SLOT + NC],
                        xt[:, idx * _P : (idx + 1) * _P],
                        coef_c[:],
                        start=True,
                        stop=True,
                    )
                ptiles.append(pt_)

            def a_copy(gi, eng):
                # copy A-halves of the group's data tiles (skip m1) into ys
                grp = groups[gi]
                pt_ = ptiles[gi]
                k0 = 1 if gi == 0 else 0
                n = len(grp) - k0
                t0 = grp[k0]
                src = pt_[:, k0 * SLOT : (k0 + n) * SLOT].rearrange(
                    "p (n w) -> p n w", w=SLOT
                )[:, :, 0:_P]
                dst = ys[:, t0 * _P : (t0 + n) * _P].rearrange(
                    "p (n w) -> p n w", w=_P
                )
                eng(dst, src)

            def b_add(gi):
                # ys[f1] += B-half(f1-1): group gi's slots feed the next tiles
                grp = groups[gi]
                if grp[-1] == 15:
                    grp = grp[:-1]      # tile 15's B-half is discarded
                    if not grp:
                        return
                pt_ = ptiles[gi]
                n = len(grp)
                tdst = 0 if gi == 0 else groups[gi][0] + 1
                src = pt_[:, 0 : n * SLOT].rearrange(
                    "p (n w) -> p n w", w=SLOT
                )[:, :, _P : _P + kb]
                dst = ys[:, tdst * _P : (tdst + n) * _P].rearrange(
                    "p (n w) -> p n w", w=_P
                )[:, :, 0:kb]
                nc.vector.tensor_add(dst, src, dst)

            act_copy = nc.scalar.copy
            dve_copy = nc.vector.tensor_copy
            gps_copy = nc.gpsimd.tensor_copy
            # A-copy engine per group: balance ACT (also does Xt copies)
            if mode == "bf16":
                a_eng = [act_copy, dve_copy, act_copy, dve_copy, act_copy]
            else:
                a_eng = [dve_copy, act_copy, dve_copy, act_copy, dve_copy]

            mm_group(0)
            a_copy(0, a_eng[0])
            for gi in range(1, 5):
                mm_group(gi)
                a_copy(gi, a_eng[gi])
                b_add(gi - 1)
            b_add(4)

            out_eng[s % len(out_eng)].dma_start(y_d.ap()[s], ys[:])

    nc.compile()
    return nc


def _make_runner(mode, kb, repeat=1):
    """Compile the bass program and wrap it in a cached shard_map'd jit."""
    import jax
    import numpy as _np
    from jax.sharding import Mesh, PartitionSpec
    from jax.experimental.shard_map import shard_map
    from concourse import bass2jax, mybir

    nc = _build_program(mode, kb, repeat)

    if os.environ.get("BIQUAD_SIM") == "1":
        def run_sim(x_all, coef):
            from concourse import bass_interp
            y_all = np.zeros_like(x_all)
            ident = np.eye(_P, dtype=np.float32)
            ncs = int(os.environ.get("BIQUAD_SIM_CORES", str(_NCORES)))
            for c in range(ncs):
                sim = bass_interp.CoreSim(nc)
                sim.tensor("x")[:] = x_all[c * _S : (c + 1) * _S]
                sim.tensor("coef")[:] = coef
                sim.tensor("ident")[:] = ident
                sim.simulate()
                y_all[c * _S : (c + 1) * _S] = sim.tensor("y")
            return y_all
        return run_sim

    bass2jax.install_neuronx_cc_hook()

    partition_name = (
        nc.partition_id_tensor.name if nc.partition_id_tensor else None
    )
    in_names, out_names, out_avals = [], [], []
    for alloc in nc.m.functions[0].allocations:
        if not isinstance(alloc, mybir.MemoryLocationSet):
            continue
        name = alloc.memorylocations[0].name
        if alloc.kind == "ExternalInput":
            if name != partition_name:
                in_names.append(name)
        elif alloc.kind == "ExternalOutput":
            out_names.append(name)
            out_avals.append(
                jax.core.ShapedArray(
                    tuple(alloc.tensor_shape), mybir.dt.np(alloc.dtype)
                )
            )
    n_params = len(in_names)
    in_names.extend(out_names)
    if partition_name is not None:
        in_names.append(partition_name)

    def _body(*args):
        operands = list(args)
        if partition_name is not None:
            operands.append(bass2jax.partition_id_tensor())
        outs = bass2jax._bass_exec_p.bind(
            *operands,
            out_avals=tuple(out_avals),
            in_names=tuple(in_names),
            out_names=tuple(out_names),
            lowering_input_output_aliases=(),
            sim_require_finite=True,
            sim_require_nnan=True,
            nc=nc,
        )
        return tuple(outs)

    devices = jax.devices()[:_NCORES]
    mesh = Mesh(_np.asarray(devices), ("core",))
    n_outs = len(out_names)
    in_specs = (PartitionSpec("core"),) * (n_params + n_outs)
    out_specs = (PartitionSpec("core"),) * n_outs
    sharded = jax.jit(
        shard_map(
            _body, mesh=mesh, in_specs=in_specs, out_specs=out_specs,
            check_rep=False,
        ),
        keep_unused=True,
    )

    name_to_idx = {n: i for i, n in enumerate(in_names[:n_params])}
    ident = np.eye(_P, dtype=np.float32)

    def run_hw(x_all, coef):
        # x_all: [64, 128, 2048] fp32; returns y_all same shape
        per_core_ins = {
            "x": x_all.reshape(_NCORES * _S, _P, _F),
            "coef": np.concatenate([coef] * _NCORES, axis=0),
            "ident": np.concatenate([ident] * _NCORES, axis=0),
        }
        args = [None] * n_params
        for n, i in name_to_idx.items():
            args[i] = per_core_ins[n]
        zeros = [
            np.zeros((_NCORES * a.shape[0], *a.shape[1:]), a.dtype)
            for a in out_avals
        ]
        outs = sharded(*args, *zeros)
        y_idx = out_names.index("y")
        return np.asarray(outs[y_idx]).reshape(_B, _P, _F)

    run_hw.sharded = sharded
    run_hw.meta = (in_names, out_names, out_avals, n_params, name_to_idx, ident)
    run_hw.nc = nc

    def make_chain():
        """Jit that runs the kernel k (runtime scalar) times back-to-back on
        device, feeding y back as x — for timing (marginal cost per step ≈
        one on-device execution). fori_loop keeps the bass_exec custom call
        appearing exactly once in the module (hook limitation), and a
        runtime k avoids recompiling per chain length."""
        x_idx = name_to_idx["x"]
        y_idx = out_names.index("y")

        def chained(k, *args):
            args = list(args)

            def body(_, x):
                a = list(args)
                a[x_idx] = x
                return _body(*a)[y_idx]

            y = jax.lax.fori_loop(0, k, body, args[x_idx])
            return (y,)

        return jax.jit(
            shard_map(
                chained, mesh=mesh,
                in_specs=(PartitionSpec(),) + in_specs,
                out_specs=(PartitionSpec("core"),),
                check_rep=False,
            ),
            keep_unused=True,
        )

    run_hw.make_chain = make_chain
    return run_hw


def _get_runner(mode, kb, repeat=1):
    key = (mode, kb, repeat, os.environ.get("BIQUAD_SIM") == "1")
    if key not in _runner_cache:
        _runner_cache[key] = _make_runner(mode, kb, repeat)
    return _runner_cache[key]


def _prepare(b0, b1, b2, a1, a2):
    """Impulse response, truncation length, coefficient block."""
    g = _impulse_response(b0, b1, b2, a1, a2, 2 * _P)
    mag = np.abs(g)
    scale = mag.max() + 1e-300
    sig = np.nonzero(mag > 1e-9 * scale)[0]
    K = int(sig[-1]) + 1 if len(sig) else 1
    if K > _P:
        raise ValueError(
            f"impulse response needs {K} taps (> {_P}); filter too close "
            "to instability for the truncated-FIR kernel"
        )
    kb = max(32, ((K + 15) // 16) * 16)   # B-half width, 16-col aligned
    coef = _coef_block(g[: _P + kb], kb)
    if _MODE == "f32r" and kb < 128:
        # pad the moving operand to NC=256 (full-rate f32r); the padded
        # B columns are zero and b_add never reads them.
        pad = np.zeros((_P, 128 - kb), dtype=np.float32)
        coef = np.concatenate([coef, pad], axis=1)
    return coef, kb


def kernel(x, b0, b1, b2, a1, a2):
    assert x.shape == (_B, _T, 1), x.shape
    coef, kb = _prepare(
        float(b0[0]), float(b1[0]), float(b2[0]), float(a1[0]), float(a2[0])
    )
    run = _get_runner(_MODE, kb)
    x_all = np.ascontiguousarray(x, dtype=np.float32).reshape(_B, _P, _F)
    y_all = run(x_all, coef)
    return y_all.reshape(_B, _T, 1)

